# revision 10
# baseline (speedup 1.0000x reference)
"""AGNNConv (single-head attention message passing) on 8 TRN2 NeuronCores.

Reference computation (N=100000 nodes, fixed degree 16, D=64):
    X_prime = X @ W                                  # [N, 64]
    e[n,k]  = <X_prime[n], X_prime[ci[n,k]]> * s     # s = attention_w[0,0]
    out[n]  = sum_k e[n,k] * X_prime[ci[n,k]]        # [N, 64]

Sharding: nodes split 12500/core across 8 cores, fully independent (no
collectives). The host pre-gathers raw X rows per edge into a transposed,
tile-ordered image XgT[f, (t,k,p)] so the device never does an irregular
gather: per tile the neighbor features d = Xg @ W are produced directly in
node-canonical PSUM layout by 16 small matmuls, and the vector engine does
the dot/weight/aggregate.
"""

import sys

import ml_dtypes
import numpy as np

if "/opt/trn_rl_repo" not in sys.path:
    sys.path.insert(0, "/opt/trn_rl_repo")

N_NODES = 100000
DEG = 16
D = 64
CORES = 8
NPC = N_NODES // CORES  # 12500
P = 128
NTILES = (NPC + P - 1) // P  # 98
NPAD = NTILES * P  # 12544


def build_nc(n_nodes=N_NODES, npc=NPC, deg=DEG, d=D, cores=CORES, lowering=False):
    from concourse import bacc, bass, mybir, tile

    ntiles = (npc + P - 1) // P
    npad = ntiles * P

    f32 = mybir.dt.float32
    bf16 = mybir.dt.bfloat16

    nc = bacc.Bacc(
        "TRN2", target_bir_lowering=lowering, debug=False, num_devices=cores
    )

    # xT carries [X_shard.T | W | W*s] so the node matmuls depend on ONE DMA.
    xT = nc.declare_dram_parameter("xT", [d, npad + 2 * d], f32, isOutput=False)
    # Pre-gathered neighbor features, transposed: XgT[f, t*2048 + k*128 + p]
    # = X[ci[t*128+p, k], f] in bf16.
    xgT = nc.declare_dram_parameter(
        "xgT", [d, ntiles * deg * P], bf16, isOutput=False
    )
    out_ext = nc.declare_dram_parameter("out", [npad, d], f32, isOutput=True)

    with tile.TileContext(nc) as tc:
        with (
            tc.tile_pool(name="const", bufs=1) as cpool,
            tc.tile_pool(name="psum", bufs=2, space="PSUM") as psum,
            tc.tile_pool(name="gpsum", bufs=3, space="PSUM") as gpsum,
            tc.tile_pool(name="xg", bufs=3) as xgpool,
            tc.tile_pool(name="prod", bufs=2) as ppool,
            tc.tile_pool(name="q", bufs=2) as qpool,
            tc.tile_pool(name="e", bufs=3) as epool,
            tc.tile_pool(name="o", bufs=3) as opool,
        ):
            xT_sb = cpool.tile([d, npad + 2 * d], f32, tag="xT_sb")
            sxp_bf = cpool.tile([P, ntiles * d], bf16, tag="sxp_bf")
            w_bf = cpool.tile([d, d], bf16, tag="w_bf")

            nc.sync.dma_start(out=xT_sb[:, :], in_=xT[:, :])
            # bf16 copy of W for the edge matmuls
            nc.vector.tensor_copy(out=w_bf[:, :], in_=xT_sb[:, npad : npad + d])

            # X_prime*s shard, one matmul per tile against W*s.
            ws = xT_sb[:, npad + d : npad + 2 * d]
            for t in range(ntiles):
                ps1 = psum.tile([P, d], f32, tag="ps1")
                nc.tensor.matmul(
                    ps1[:, :],
                    xT_sb[:, t * P : (t + 1) * P],
                    ws,
                    start=True,
                    stop=True,
                )
                nc.vector.tensor_copy(
                    out=sxp_bf[:, t * d : (t + 1) * d], in_=ps1[:, :]
                )

            # Edge phase: stream XgT tile, 16 matmuls -> G in PSUM
            # (node-canonical layout [p, k*64+f]); Act converts to bf16;
            # the dot/weight/aggregate chain alternates DVE / GpSimd by tile.
            for t in range(ntiles):
                rows = min(P, npc - t * P)
                ve = nc.vector
                xg_t = xgpool.tile([d, deg * P], bf16, tag="xg_t")
                nc.sync.dma_start(
                    out=xg_t[:, :],
                    in_=xgT[:, t * deg * P : (t + 1) * deg * P],
                )
                Gp = gpsum.tile([P, deg * d], f32, tag="Gp")
                for k in range(deg):
                    nc.tensor.matmul(
                        Gp[:, k * d : (k + 1) * d],
                        xg_t[:, k * P : (k + 1) * P],
                        w_bf[:, :],
                        start=True,
                        stop=True,
                    )
                Gb = ppool.tile([P, deg * d], bf16, tag="Gb")
                nc.scalar.copy(out=Gb[0:rows, :], in_=Gp[0:rows, :])
                Gv = Gb[0:rows, :].rearrange("p (k f) -> p k f", k=deg)
                Pt = ppool.tile([P, deg * d], bf16, tag="Pt")
                nc.gpsimd.tensor_tensor(
                    out=Pt[0:rows, :].rearrange("p (k f) -> p k f", k=deg),
                    in0=Gv,
                    in1=sxp_bf[0:rows, t * d : (t + 1) * d]
                    .unsqueeze(1)
                    .broadcast_to([rows, deg, d]),
                    op=mybir.AluOpType.mult,
                )
                e = epool.tile([P, deg], bf16, tag="e")
                with nc.allow_low_precision(reason="bf16 edge attn within tolerance"):
                    ve.tensor_reduce(
                        out=e[0:rows, :],
                        in_=Pt[0:rows, :].rearrange("p (k f) -> p k f", k=deg),
                        axis=mybir.AxisListType.X,
                        op=mybir.AluOpType.add,
                    )
                Qt = qpool.tile([P, deg * d], bf16, tag="Qt")
                ve.tensor_tensor(
                    out=Qt[0:rows, :].rearrange("p (k f) -> p k f", k=deg),
                    in0=Gv,
                    in1=e[0:rows, :].unsqueeze(2).broadcast_to([rows, deg, d]),
                    op=mybir.AluOpType.mult,
                )
                # o = sum_k Qt[:, k, :] -- two tree levels on GpSimd
                # (contiguous halves), final strided reduce on DVE.
                A1 = qpool.tile([P, deg * d // 2], bf16, tag="A1")
                nc.gpsimd.tensor_tensor(
                    out=A1[0:rows, :],
                    in0=Qt[0:rows, 0 : deg * d // 2],
                    in1=Qt[0:rows, deg * d // 2 :],
                    op=mybir.AluOpType.add,
                )
                A2 = qpool.tile([P, deg * d // 4], bf16, tag="A2")
                nc.gpsimd.tensor_tensor(
                    out=A2[0:rows, :],
                    in0=A1[0:rows, 0 : deg * d // 4],
                    in1=A1[0:rows, deg * d // 4 :],
                    op=mybir.AluOpType.add,
                )
                o = opool.tile([P, d], f32, tag="o")
                ve.tensor_reduce(
                    out=o[0:rows, :],
                    in_=A2[0:rows, :].rearrange("p (k f) -> p f k", k=deg // 4),
                    axis=mybir.AxisListType.X,
                    op=mybir.AluOpType.add,
                )
                nc.sync.dma_start(
                    out=out_ext[t * P : t * P + rows, :], in_=o[0:rows, :]
                )

    nc.compile()
    return nc


def make_in_maps(X, weights, attention_w, column_index, n_nodes=N_NODES, cores=CORES):
    npc = n_nodes // cores
    ntiles = (npc + P - 1) // P
    npad = ntiles * P
    s = float(np.asarray(attention_w).reshape(-1)[0])
    w = np.asarray(weights, dtype=np.float32)
    Xf = np.asarray(X, dtype=np.float32)
    XbfT = np.ascontiguousarray(Xf.astype(ml_dtypes.bfloat16).T)  # [64, N]
    ci_all = np.asarray(column_index, dtype=np.int64).reshape(n_nodes, DEG)
    in_maps = []
    for c in range(cores):
        r0, r1 = c * npc, (c + 1) * npc
        xT = np.zeros((D, npad + 2 * D), dtype=np.float32)
        xT[:, :npc] = Xf[r0:r1].T
        xT[:, npad : npad + D] = w
        xT[:, npad + D : npad + 2 * D] = w * s
        ci_pad = np.zeros((npad, DEG), dtype=np.int64)
        ci_pad[:npc] = ci_all[r0:r1]
        # slot order (t, k, p)
        perm = ci_pad.reshape(ntiles, P, DEG).transpose(0, 2, 1).reshape(-1)
        xgT = XbfT[:, perm]  # [64, ntiles*deg*P]
        in_maps.append(
            {
                "xT": np.ascontiguousarray(xT),
                "xgT": np.ascontiguousarray(xgT),
            }
        )
    return in_maps


_NC_CACHE = {}


def _get_nc():
    key = (N_NODES, NPC)
    if key not in _NC_CACHE:
        _NC_CACHE[key] = build_nc()
    return _NC_CACHE[key]


def run(X, weights, attention_w, column_index, trace=False, **trace_kwargs):
    from concourse import bass_utils

    nc = _get_nc()
    in_maps = make_in_maps(X, weights, attention_w, column_index)
    res = bass_utils.run_bass_kernel_spmd(
        nc, in_maps, core_ids=list(range(CORES)), trace=trace, **trace_kwargs
    )
    outs = [np.asarray(res.results[c]["out"][:NPC]) for c in range(CORES)]
    return np.concatenate(outs, axis=0).astype(np.float32), res


def kernel(
    X,
    weights,
    attention_w,
    row_pointers,
    column_index,
    blockPartition,
    edgeToColumn,
    edgeToRow,
    **_unused,
):
    out, _ = run(X, weights, attention_w, column_index)
    return out


# revision 13
# speedup vs baseline: 1.2538x; 1.2538x over previous
"""AGNNConv (single-head attention message passing) on 8 TRN2 NeuronCores.

Reference computation (N=100000 nodes, fixed degree 16, D=64):
    X_prime = X @ W                                  # [N, 64]
    e[n,k]  = <X_prime[n], X_prime[ci[n,k]]> * s     # s = attention_w[0,0]
    out[n]  = sum_k e[n,k] * X_prime[ci[n,k]]        # [N, 64]

Sharding: nodes split 12500/core across 8 cores, fully independent.
The host pre-gathers raw X rows per edge. Everything on device runs
feature-major with TWO tiles packed on the 128 partitions (tile A's 64
features on partitions 0-63, tile B's on 64-127):

  per tile pair (2 x 128 nodes, 4096 edge slots, slot s = p*16+k):
    D^T   = blockdiag(W,W)^T-matmul over pre-gathered XgT2   (tensor)
    P2    = D^T * xs2 (node features, k-broadcast)           (DVE)
    e_br  = blockdiag(ones)-matmul over P2 -> per-slot dots
            replicated across each tile's 64 partitions      (tensor)
    Qt    = D^T * e_br                                       (GpSimd)
    o^T   = sum_k Qt  (k contiguous innermost)               (DVE)

so the per-edge dot products and their broadcast both ride the idle
tensor engine instead of the vector engine.
"""

import sys

import ml_dtypes
import numpy as np

if "/opt/trn_rl_repo" not in sys.path:
    sys.path.insert(0, "/opt/trn_rl_repo")

N_NODES = 100000
DEG = 16
D = 64
CORES = 8
NPC = N_NODES // CORES  # 12500
P = 128
NTILES = (NPC + P - 1) // P  # 98
NPAIRS = NTILES // 2  # 49
SLOTS = P * DEG  # 2048 slots per tile


def build_nc(lowering=False):
    from concourse import bacc, mybir, tile

    f32 = mybir.dt.float32
    bf16 = mybir.dt.bfloat16

    nc = bacc.Bacc(
        "TRN2", target_bir_lowering=lowering, debug=False, num_devices=CORES
    )

    # xT2: stacked-pair node features [f + 64*(t%2), pair*128 + p] plus the
    # blockdiag(W*s, W*s) stationary in the last 128 columns.
    xT2 = nc.declare_dram_parameter(
        "xT2", [P, NPAIRS * P + P], f32, isOutput=False
    )
    # cst: [Wb2 | J]  (blockdiag(W,W), blockdiag(ones64,ones64)), bf16.
    cst = nc.declare_dram_parameter("cst", [P, 2 * P], bf16, isOutput=False)
    # Pre-gathered neighbor features, stacked-pair feature-major:
    # xgT2[f + 64*(t%2), pair*2048 + p*16 + k] = X[ci[t*128+p, k], f]
    xgT2 = nc.declare_dram_parameter(
        "xgT2", [P, NPAIRS * SLOTS], bf16, isOutput=False
    )
    out_ext = nc.declare_dram_parameter("out", [P, NPAIRS * P], f32, isOutput=True)

    CH = 512  # psum bank chunk (f32)

    with tile.TileContext(nc) as tc:
        with (
            tc.tile_pool(name="const", bufs=1) as cpool,
            tc.tile_pool(name="dps", bufs=3, space="PSUM") as dpsum,
            tc.tile_pool(name="eps", bufs=3, space="PSUM") as epsum,
            tc.tile_pool(name="xg", bufs=3) as xgpool,
            tc.tile_pool(name="db", bufs=2) as dbpool,
            tc.tile_pool(name="p2", bufs=2) as p2pool,
            tc.tile_pool(name="qt", bufs=2) as qtpool,
            tc.tile_pool(name="o", bufs=3) as opool,
        ):
            xT2_sb = cpool.tile([P, NPAIRS * P + P], f32, tag="xT2_sb")
            cst_sb = cpool.tile([P, 2 * P], bf16, tag="cst_sb")
            xs2_sb = cpool.tile([P, NPAIRS * P], bf16, tag="xs2_sb")

            nc.sync.dma_start(out=xT2_sb[:, :], in_=xT2[:, :])
            nc.sync.dma_start(out=cst_sb[:, :], in_=cst[:, :])
            w2s = xT2_sb[:, NPAIRS * P : NPAIRS * P + P]
            wb2 = cst_sb[:, 0:P]
            jj = cst_sb[:, P : 2 * P]

            # Node phase: xs2 = blockdiag(W*s,W*s)^T @ X2T, all pairs first
            # (single stationary load), Act converts PSUM -> bf16.
            for pr in range(NPAIRS):
                ps = dpsum.tile([P, CH], f32, tag="D")
                nc.tensor.matmul(
                    ps[:, 0:P],
                    w2s,
                    xT2_sb[:, pr * P : (pr + 1) * P],
                    start=True,
                    stop=True,
                )
                nc.scalar.copy(
                    out=xs2_sb[:, pr * P : (pr + 1) * P], in_=ps[:, 0:P]
                )

            # Edge phase
            for pr in range(NPAIRS):
                xg = xgpool.tile([P, SLOTS], bf16, tag="xg")
                nc.sync.dma_start(
                    out=xg[:, :], in_=xgT2[:, pr * SLOTS : (pr + 1) * SLOTS]
                )
                # D^T in 4 bank chunks; Act converts each to bf16 SBUF
                Db = dbpool.tile([P, SLOTS], bf16, tag="Db")
                for j in range(4):
                    Dp = dpsum.tile([P, CH], f32, tag="D")
                    nc.tensor.matmul(
                        Dp[:, :],
                        wb2,
                        xg[:, j * CH : (j + 1) * CH],
                        start=True,
                        stop=True,
                    )
                    nc.scalar.copy(
                        out=Db[:, j * CH : (j + 1) * CH], in_=Dp[:, :]
                    )
                # P2 = D * xs2 (bcast over k, 16 inner), full pair width
                P2 = p2pool.tile([P, SLOTS], bf16, tag="P2")
                nc.vector.tensor_tensor(
                    out=P2[:, :].rearrange("q (p k) -> q p k", k=DEG),
                    in0=Db[:, :].rearrange("q (p k) -> q p k", k=DEG),
                    in1=xs2_sb[:, pr * P : (pr + 1) * P]
                    .unsqueeze(2)
                    .broadcast_to([P, P, DEG]),
                    op=mybir.AluOpType.mult,
                )
                # e_br = blockdiag(ones) @ P2; Act stages it to SBUF bf16,
                # then Qt = D * e_br full width on GpSimd.
                Eb = p2pool.tile([P, SLOTS], bf16, tag="Eb")
                for j in range(4):
                    Ep = epsum.tile([P, CH], f32, tag="E")
                    nc.tensor.matmul(
                        Ep[:, :],
                        jj,
                        P2[:, j * CH : (j + 1) * CH],
                        start=True,
                        stop=True,
                    )
                    nc.scalar.copy(
                        out=Eb[:, j * CH : (j + 1) * CH], in_=Ep[:, :]
                    )
                Qt = qtpool.tile([P, SLOTS], bf16, tag="Qt")
                nc.gpsimd.tensor_tensor(
                    out=Qt[:, :],
                    in0=Db[:, :],
                    in1=Eb[:, :],
                    op=mybir.AluOpType.mult,
                )
                o2 = opool.tile([P, P], f32, tag="o2")
                nc.vector.tensor_reduce(
                    out=o2[:, :],
                    in_=Qt[:, :].rearrange("q (p k) -> q p k", k=DEG),
                    axis=mybir.AxisListType.X,
                    op=mybir.AluOpType.add,
                )
                nc.sync.dma_start(
                    out=out_ext[:, pr * P : (pr + 1) * P], in_=o2[:, :]
                )

    nc.compile()
    return nc


def make_in_maps(X, weights, attention_w, column_index):
    s = float(np.asarray(attention_w).reshape(-1)[0])
    w = np.asarray(weights, dtype=np.float32)
    Xf = np.asarray(X, dtype=np.float32)
    Xbf = Xf.astype(ml_dtypes.bfloat16)
    ci_all = np.asarray(column_index, dtype=np.int64).reshape(N_NODES, DEG)
    NPAD = NTILES * P

    ws = w * s
    w2s = np.zeros((P, P), dtype=np.float32)
    w2s[0:D, 0:D] = ws
    w2s[D:P, D:P] = ws
    wb2 = np.zeros((P, P), dtype=ml_dtypes.bfloat16)
    wb2[0:D, 0:D] = w.astype(ml_dtypes.bfloat16)
    wb2[D:P, D:P] = w.astype(ml_dtypes.bfloat16)
    jmat = np.zeros((P, P), dtype=ml_dtypes.bfloat16)
    jmat[0:D, 0:D] = 1
    jmat[D:P, D:P] = 1
    cst = np.concatenate([wb2, jmat], axis=1)

    in_maps = []
    for c in range(CORES):
        r0 = c * NPC
        Xsh = np.zeros((NPAD, D), dtype=np.float32)
        Xsh[:NPC] = Xf[r0 : r0 + NPC]
        # stacked pairs: [f + 64*(t%2), pair*128 + p]
        x4 = Xsh.reshape(NPAIRS, 2, P, D)  # [pair, tpar, p, f]
        xT2 = np.zeros((P, NPAIRS * P + P), dtype=np.float32)
        xT2[:, : NPAIRS * P] = (
            x4.transpose(1, 3, 0, 2).reshape(2 * D, NPAIRS * P)
        )
        xT2[:, NPAIRS * P :] = w2s

        ci_pad = np.zeros((NPAD, DEG), dtype=np.int64)
        ci_pad[:NPC] = ci_all[r0 : r0 + NPC]
        # xgT2[f + 64*tp, pair*2048 + p*16 + k]
        g = Xbf[ci_pad, :]  # [NPAD, DEG, D]
        g5 = g.reshape(NPAIRS, 2, P, DEG, D)  # [pair, tp, p, k, f]
        xgT2 = np.ascontiguousarray(
            g5.transpose(1, 4, 0, 2, 3).reshape(2 * D, NPAIRS * SLOTS)
        )
        in_maps.append(
            {
                "xT2": np.ascontiguousarray(xT2),
                "cst": np.ascontiguousarray(cst),
                "xgT2": xgT2,
            }
        )
    return in_maps


_NC_CACHE = {}


def _get_nc():
    if "nc" not in _NC_CACHE:
        _NC_CACHE["nc"] = build_nc()
    return _NC_CACHE["nc"]


def run(X, weights, attention_w, column_index, trace=False, **trace_kwargs):
    from concourse import bass_utils

    nc = _get_nc()
    in_maps = make_in_maps(X, weights, attention_w, column_index)
    res = bass_utils.run_bass_kernel_spmd(
        nc, in_maps, core_ids=list(range(CORES)), trace=trace, **trace_kwargs
    )
    outs = []
    for c in range(CORES):
        o = np.asarray(res.results[c]["out"])  # [128, NPAIRS*128]
        # out[f + 64*tp, pair*128 + p] -> [node, f]
        o4 = o.reshape(2, D, NPAIRS, P).transpose(2, 0, 3, 1).reshape(NTILES * P, D)
        outs.append(o4[:NPC])
    return np.concatenate(outs, axis=0).astype(np.float32), res


def kernel(
    X,
    weights,
    attention_w,
    row_pointers,
    column_index,
    blockPartition,
    edgeToColumn,
    edgeToRow,
    **_unused,
):
    out, _ = run(X, weights, attention_w, column_index)
    return out


# revision 14
# speedup vs baseline: 1.4190x; 1.1318x over previous
"""AGNNConv (single-head attention message passing) on 8 TRN2 NeuronCores.

Reference computation (N=100000 nodes, fixed degree 16, D=64):
    X_prime = X @ W                                  # [N, 64]
    e[n,k]  = <X_prime[n], X_prime[ci[n,k]]> * s     # s = attention_w[0,0]
    out[n]  = sum_k e[n,k] * X_prime[ci[n,k]]        # [N, 64]

Sharding: nodes split 12500/core across 8 cores, fully independent.
The host pre-gathers raw X rows per edge. Everything on device runs
feature-major with TWO tiles packed on the 128 partitions (tile A's 64
features on partitions 0-63, tile B's on 64-127):

  per tile pair (2 x 128 nodes, 4096 edge slots, slot s = p*16+k):
    D^T   = blockdiag(W,W)^T-matmul over pre-gathered XgT2   (tensor)
    P2    = D^T * xs2 (node features, k-broadcast)           (DVE)
    e_br  = blockdiag(ones)-matmul over P2 -> per-slot dots
            replicated across each tile's 64 partitions      (tensor)
    Qt    = D^T * e_br                                       (GpSimd)
    o^T   = sum_k Qt  (k contiguous innermost)               (DVE)

so the per-edge dot products and their broadcast both ride the idle
tensor engine instead of the vector engine.
"""

import sys

import ml_dtypes
import numpy as np

if "/opt/trn_rl_repo" not in sys.path:
    sys.path.insert(0, "/opt/trn_rl_repo")

N_NODES = 100000
DEG = 16
D = 64
CORES = 8
NPC = N_NODES // CORES  # 12500
P = 128
NTILES = (NPC + P - 1) // P  # 98
NPAIRS = NTILES // 2  # 49
SLOTS = P * DEG  # 2048 slots per tile


def build_nc(lowering=False):
    from concourse import bacc, mybir, tile

    f32 = mybir.dt.float32
    bf16 = mybir.dt.bfloat16

    nc = bacc.Bacc(
        "TRN2", target_bir_lowering=lowering, debug=False, num_devices=CORES
    )

    # xT2: stacked-pair node features [f + 64*(t%2), pair*128 + p] plus the
    # blockdiag(W*s, W*s) stationary in the last 128 columns.
    xT2 = nc.declare_dram_parameter(
        "xT2", [P, NPAIRS * P + P], f32, isOutput=False
    )
    # cst: [Wb2 | J]  (blockdiag(W,W), blockdiag(ones64,ones64)), bf16.
    cst = nc.declare_dram_parameter("cst", [P, 2 * P], bf16, isOutput=False)
    # Pre-gathered neighbor features, stacked-pair feature-major:
    # xgT2[f + 64*(t%2), pair*2048 + p*16 + k] = X[ci[t*128+p, k], f]
    xgT2 = nc.declare_dram_parameter(
        "xgT2", [P, NPAIRS * SLOTS], bf16, isOutput=False
    )
    out_ext = nc.declare_dram_parameter("out", [P, NPAIRS * P], f32, isOutput=True)

    CH = 512  # psum bank chunk (f32)

    with tile.TileContext(nc) as tc:
        with (
            tc.tile_pool(name="const", bufs=1) as cpool,
            tc.tile_pool(name="dps", bufs=4, space="PSUM") as dpsum,
            tc.tile_pool(name="eps", bufs=4, space="PSUM") as epsum,
            tc.tile_pool(name="xg", bufs=4) as xgpool,
            tc.tile_pool(name="db", bufs=3) as dbpool,
            tc.tile_pool(name="p2", bufs=3) as p2pool,
            tc.tile_pool(name="qt", bufs=3) as qtpool,
            tc.tile_pool(name="o", bufs=4) as opool,
        ):
            xT2_sb = cpool.tile([P, NPAIRS * P + P], f32, tag="xT2_sb")
            cst_sb = cpool.tile([P, 2 * P], bf16, tag="cst_sb")
            xs2_sb = cpool.tile([P, NPAIRS * P], bf16, tag="xs2_sb")

            nc.sync.dma_start(out=xT2_sb[:, :], in_=xT2[:, :])
            nc.sync.dma_start(out=cst_sb[:, :], in_=cst[:, :])
            w2s = xT2_sb[:, NPAIRS * P : NPAIRS * P + P]
            wb2 = cst_sb[:, 0:P]
            jj = cst_sb[:, P : 2 * P]

            # Node phase: xs2 = blockdiag(W*s,W*s)^T @ X2T, all pairs first
            # (single stationary load), Act converts PSUM -> bf16.
            for pr in range(NPAIRS):
                ps = dpsum.tile([P, CH], f32, tag="D")
                nc.tensor.matmul(
                    ps[:, 0:P],
                    w2s,
                    xT2_sb[:, pr * P : (pr + 1) * P],
                    start=True,
                    stop=True,
                )
                nc.scalar.copy(
                    out=xs2_sb[:, pr * P : (pr + 1) * P], in_=ps[:, 0:P]
                )

            # Edge phase
            for pr in range(NPAIRS):
                xg = xgpool.tile([P, SLOTS], bf16, tag="xg")
                nc.sync.dma_start(
                    out=xg[:, :], in_=xgT2[:, pr * SLOTS : (pr + 1) * SLOTS]
                )
                # D^T in 4 bank chunks; Act converts each to bf16 SBUF
                Db = dbpool.tile([P, SLOTS], bf16, tag="Db")
                for j in range(4):
                    Dp = dpsum.tile([P, CH], f32, tag="D")
                    nc.tensor.matmul(
                        Dp[:, :],
                        wb2,
                        xg[:, j * CH : (j + 1) * CH],
                        start=True,
                        stop=True,
                    )
                    nc.scalar.copy(
                        out=Db[:, j * CH : (j + 1) * CH], in_=Dp[:, :]
                    )
                # P2 = D * xs2 (bcast over k, 16 inner), full pair width
                P2 = p2pool.tile([P, SLOTS], bf16, tag="P2")
                nc.vector.tensor_tensor(
                    out=P2[:, :].rearrange("q (p k) -> q p k", k=DEG),
                    in0=Db[:, :].rearrange("q (p k) -> q p k", k=DEG),
                    in1=xs2_sb[:, pr * P : (pr + 1) * P]
                    .unsqueeze(2)
                    .broadcast_to([P, P, DEG]),
                    op=mybir.AluOpType.mult,
                )
                # e_br = blockdiag(ones) @ P2; Act stages it to SBUF bf16,
                # then Qt = D * e_br full width on GpSimd.
                Eb = p2pool.tile([P, SLOTS], bf16, tag="Eb")
                for j in range(4):
                    Ep = epsum.tile([P, CH], f32, tag="E")
                    nc.tensor.matmul(
                        Ep[:, :],
                        jj,
                        P2[:, j * CH : (j + 1) * CH],
                        start=True,
                        stop=True,
                    )
                    nc.scalar.copy(
                        out=Eb[:, j * CH : (j + 1) * CH], in_=Ep[:, :]
                    )
                Qt = qtpool.tile([P, SLOTS], bf16, tag="Qt")
                nc.gpsimd.tensor_tensor(
                    out=Qt[:, :],
                    in0=Db[:, :],
                    in1=Eb[:, :],
                    op=mybir.AluOpType.mult,
                )
                o2 = opool.tile([P, P], f32, tag="o2")
                nc.vector.tensor_reduce(
                    out=o2[:, :],
                    in_=Qt[:, :].rearrange("q (p k) -> q p k", k=DEG),
                    axis=mybir.AxisListType.X,
                    op=mybir.AluOpType.add,
                )
                nc.sync.dma_start(
                    out=out_ext[:, pr * P : (pr + 1) * P], in_=o2[:, :]
                )

    nc.compile()
    return nc


def make_in_maps(X, weights, attention_w, column_index):
    s = float(np.asarray(attention_w).reshape(-1)[0])
    w = np.asarray(weights, dtype=np.float32)
    Xf = np.asarray(X, dtype=np.float32)
    Xbf = Xf.astype(ml_dtypes.bfloat16)
    ci_all = np.asarray(column_index, dtype=np.int64).reshape(N_NODES, DEG)
    NPAD = NTILES * P

    ws = w * s
    w2s = np.zeros((P, P), dtype=np.float32)
    w2s[0:D, 0:D] = ws
    w2s[D:P, D:P] = ws
    wb2 = np.zeros((P, P), dtype=ml_dtypes.bfloat16)
    wb2[0:D, 0:D] = w.astype(ml_dtypes.bfloat16)
    wb2[D:P, D:P] = w.astype(ml_dtypes.bfloat16)
    jmat = np.zeros((P, P), dtype=ml_dtypes.bfloat16)
    jmat[0:D, 0:D] = 1
    jmat[D:P, D:P] = 1
    cst = np.concatenate([wb2, jmat], axis=1)

    in_maps = []
    for c in range(CORES):
        r0 = c * NPC
        Xsh = np.zeros((NPAD, D), dtype=np.float32)
        Xsh[:NPC] = Xf[r0 : r0 + NPC]
        # stacked pairs: [f + 64*(t%2), pair*128 + p]
        x4 = Xsh.reshape(NPAIRS, 2, P, D)  # [pair, tpar, p, f]
        xT2 = np.zeros((P, NPAIRS * P + P), dtype=np.float32)
        xT2[:, : NPAIRS * P] = (
            x4.transpose(1, 3, 0, 2).reshape(2 * D, NPAIRS * P)
        )
        xT2[:, NPAIRS * P :] = w2s

        ci_pad = np.zeros((NPAD, DEG), dtype=np.int64)
        ci_pad[:NPC] = ci_all[r0 : r0 + NPC]
        # xgT2[f + 64*tp, pair*2048 + p*16 + k]
        g = Xbf[ci_pad, :]  # [NPAD, DEG, D]
        g5 = g.reshape(NPAIRS, 2, P, DEG, D)  # [pair, tp, p, k, f]
        xgT2 = np.ascontiguousarray(
            g5.transpose(1, 4, 0, 2, 3).reshape(2 * D, NPAIRS * SLOTS)
        )
        in_maps.append(
            {
                "xT2": np.ascontiguousarray(xT2),
                "cst": np.ascontiguousarray(cst),
                "xgT2": xgT2,
            }
        )
    return in_maps


_NC_CACHE = {}


def _get_nc():
    if "nc" not in _NC_CACHE:
        _NC_CACHE["nc"] = build_nc()
    return _NC_CACHE["nc"]


def run(X, weights, attention_w, column_index, trace=False, **trace_kwargs):
    from concourse import bass_utils

    nc = _get_nc()
    in_maps = make_in_maps(X, weights, attention_w, column_index)
    res = bass_utils.run_bass_kernel_spmd(
        nc, in_maps, core_ids=list(range(CORES)), trace=trace, **trace_kwargs
    )
    outs = []
    for c in range(CORES):
        o = np.asarray(res.results[c]["out"])  # [128, NPAIRS*128]
        # out[f + 64*tp, pair*128 + p] -> [node, f]
        o4 = o.reshape(2, D, NPAIRS, P).transpose(2, 0, 3, 1).reshape(NTILES * P, D)
        outs.append(o4[:NPC])
    return np.concatenate(outs, axis=0).astype(np.float32), res


def kernel(
    X,
    weights,
    attention_w,
    row_pointers,
    column_index,
    blockPartition,
    edgeToColumn,
    edgeToRow,
    **_unused,
):
    out, _ = run(X, weights, attention_w, column_index)
    return out


# revision 15
# speedup vs baseline: 1.4209x; 1.0014x over previous
"""AGNNConv (single-head attention message passing) on 8 TRN2 NeuronCores.

Reference computation (N=100000 nodes, fixed degree 16, D=64):
    X_prime = X @ W                                  # [N, 64]
    e[n,k]  = <X_prime[n], X_prime[ci[n,k]]> * s     # s = attention_w[0,0]
    out[n]  = sum_k e[n,k] * X_prime[ci[n,k]]        # [N, 64]

Sharding: nodes split 12500/core across 8 cores, fully independent.
The host pre-gathers raw X rows per edge. Everything on device runs
feature-major with TWO tiles packed on the 128 partitions (tile A's 64
features on partitions 0-63, tile B's on 64-127):

  per tile pair (2 x 128 nodes, 4096 edge slots, slot s = p*16+k):
    D^T   = blockdiag(W,W)^T-matmul over pre-gathered XgT2   (tensor)
    P2    = D^T * xs2 (node features, k-broadcast)           (DVE)
    e_br  = blockdiag(ones)-matmul over P2 -> per-slot dots
            replicated across each tile's 64 partitions      (tensor)
    Qt    = D^T * e_br                                       (GpSimd)
    o^T   = sum_k Qt  (k contiguous innermost)               (DVE)

so the per-edge dot products and their broadcast both ride the idle
tensor engine instead of the vector engine.
"""

import sys

import ml_dtypes
import numpy as np

if "/opt/trn_rl_repo" not in sys.path:
    sys.path.insert(0, "/opt/trn_rl_repo")

N_NODES = 100000
DEG = 16
D = 64
CORES = 8
NPC = N_NODES // CORES  # 12500
P = 128
NTILES = (NPC + P - 1) // P  # 98
NPAIRS = NTILES // 2  # 49
SLOTS = P * DEG  # 2048 slots per tile


def build_nc(lowering=False):
    from concourse import bacc, mybir, tile

    f32 = mybir.dt.float32
    bf16 = mybir.dt.bfloat16

    nc = bacc.Bacc(
        "TRN2", target_bir_lowering=lowering, debug=False, num_devices=CORES
    )

    # xT2: stacked-pair node features [f + 64*(t%2), pair*128 + p] plus the
    # blockdiag(W*s, W*s) stationary in the last 128 columns.
    xT2 = nc.declare_dram_parameter(
        "xT2", [P, NPAIRS * P + P], f32, isOutput=False
    )
    # cst: [Wb2 | J]  (blockdiag(W,W), blockdiag(ones64,ones64)), bf16.
    cst = nc.declare_dram_parameter("cst", [P, 2 * P], bf16, isOutput=False)
    # Pre-gathered neighbor features, stacked-pair feature-major:
    # xgT2[f + 64*(t%2), pair*2048 + p*16 + k] = X[ci[t*128+p, k], f]
    xgT2 = nc.declare_dram_parameter(
        "xgT2", [P, NPAIRS * SLOTS], bf16, isOutput=False
    )
    out_ext = nc.declare_dram_parameter("out", [P, NPAIRS * P], f32, isOutput=True)

    CH = 512  # psum bank chunk (f32)

    with tile.TileContext(nc) as tc:
        with (
            tc.tile_pool(name="const", bufs=1) as cpool,
            tc.tile_pool(name="dps", bufs=4, space="PSUM") as dpsum,
            tc.tile_pool(name="eps", bufs=4, space="PSUM") as epsum,
            tc.tile_pool(name="xg", bufs=4) as xgpool,
            tc.tile_pool(name="db", bufs=3) as dbpool,
            tc.tile_pool(name="p2", bufs=3) as p2pool,
            tc.tile_pool(name="qt", bufs=3) as qtpool,
            tc.tile_pool(name="o", bufs=4) as opool,
        ):
            xT2_sb = cpool.tile([P, NPAIRS * P + P], f32, tag="xT2_sb")
            cst_sb = cpool.tile([P, 2 * P], bf16, tag="cst_sb")
            xs2_sb = cpool.tile([P, NPAIRS * P], bf16, tag="xs2_sb")

            nc.sync.dma_start(out=xT2_sb[:, :], in_=xT2[:, :])
            nc.sync.dma_start(out=cst_sb[:, :], in_=cst[:, :])
            w2s = xT2_sb[:, NPAIRS * P : NPAIRS * P + P]
            wb2 = cst_sb[:, 0:P]
            jj = cst_sb[:, P : 2 * P]

            # Node phase: xs2 = blockdiag(W*s,W*s)^T @ X2T, all pairs first
            # (single stationary load), Act converts PSUM -> bf16.
            for pr in range(NPAIRS):
                ps = dpsum.tile([P, CH], f32, tag="D")
                nc.tensor.matmul(
                    ps[:, 0:P],
                    w2s,
                    xT2_sb[:, pr * P : (pr + 1) * P],
                    start=True,
                    stop=True,
                )
                nc.scalar.copy(
                    out=xs2_sb[:, pr * P : (pr + 1) * P], in_=ps[:, 0:P]
                )

            # Edge phase
            for pr in range(NPAIRS):
                xg = xgpool.tile([P, SLOTS], bf16, tag="xg")
                nc.sync.dma_start(
                    out=xg[:, :], in_=xgT2[:, pr * SLOTS : (pr + 1) * SLOTS]
                )
                Db = dbpool.tile([P, SLOTS], bf16, tag="Db")
                P2 = p2pool.tile([P, SLOTS], bf16, tag="P2")
                Eb = p2pool.tile([P, SLOTS], bf16, tag="Eb")
                Qt = qtpool.tile([P, SLOTS], bf16, tag="Qt")
                o2 = opool.tile([P, P], f32, tag="o2")
                HS = SLOTS // 2  # 1024-slot half-chain (p 0-63 / 64-127)
                for h in range(2):
                    # D^T in 512 bank chunks; Act converts each to bf16 SBUF
                    for j in range(2 * h, 2 * h + 2):
                        Dp = dpsum.tile([P, CH], f32, tag="D")
                        nc.tensor.matmul(
                            Dp[:, :],
                            wb2,
                            xg[:, j * CH : (j + 1) * CH],
                            start=True,
                            stop=True,
                        )
                        nc.scalar.copy(
                            out=Db[:, j * CH : (j + 1) * CH], in_=Dp[:, :]
                        )
                    # P2 = D * xs2 (bcast over k, 16 inner)
                    nc.vector.tensor_tensor(
                        out=P2[:, h * HS : (h + 1) * HS].rearrange(
                            "q (p k) -> q p k", k=DEG
                        ),
                        in0=Db[:, h * HS : (h + 1) * HS].rearrange(
                            "q (p k) -> q p k", k=DEG
                        ),
                        in1=xs2_sb[:, pr * P + h * 64 : pr * P + (h + 1) * 64]
                        .unsqueeze(2)
                        .broadcast_to([P, 64, DEG]),
                        op=mybir.AluOpType.mult,
                    )
                    # e_br = blockdiag(ones) @ P2; Act stages to SBUF bf16
                    for j in range(2 * h, 2 * h + 2):
                        Ep = epsum.tile([P, CH], f32, tag="E")
                        nc.tensor.matmul(
                            Ep[:, :],
                            jj,
                            P2[:, j * CH : (j + 1) * CH],
                            start=True,
                            stop=True,
                        )
                        nc.scalar.copy(
                            out=Eb[:, j * CH : (j + 1) * CH], in_=Ep[:, :]
                        )
                    # Qt = D * e_br on GpSimd
                    nc.gpsimd.tensor_tensor(
                        out=Qt[:, h * HS : (h + 1) * HS],
                        in0=Db[:, h * HS : (h + 1) * HS],
                        in1=Eb[:, h * HS : (h + 1) * HS],
                        op=mybir.AluOpType.mult,
                    )
                    nc.vector.tensor_reduce(
                        out=o2[:, h * 64 : (h + 1) * 64],
                        in_=Qt[:, h * HS : (h + 1) * HS].rearrange(
                            "q (p k) -> q p k", k=DEG
                        ),
                        axis=mybir.AxisListType.X,
                        op=mybir.AluOpType.add,
                    )
                nc.sync.dma_start(
                    out=out_ext[:, pr * P : (pr + 1) * P], in_=o2[:, :]
                )

    nc.compile()
    return nc


def make_in_maps(X, weights, attention_w, column_index):
    s = float(np.asarray(attention_w).reshape(-1)[0])
    w = np.asarray(weights, dtype=np.float32)
    Xf = np.asarray(X, dtype=np.float32)
    Xbf = Xf.astype(ml_dtypes.bfloat16)
    ci_all = np.asarray(column_index, dtype=np.int64).reshape(N_NODES, DEG)
    NPAD = NTILES * P

    ws = w * s
    w2s = np.zeros((P, P), dtype=np.float32)
    w2s[0:D, 0:D] = ws
    w2s[D:P, D:P] = ws
    wb2 = np.zeros((P, P), dtype=ml_dtypes.bfloat16)
    wb2[0:D, 0:D] = w.astype(ml_dtypes.bfloat16)
    wb2[D:P, D:P] = w.astype(ml_dtypes.bfloat16)
    jmat = np.zeros((P, P), dtype=ml_dtypes.bfloat16)
    jmat[0:D, 0:D] = 1
    jmat[D:P, D:P] = 1
    cst = np.concatenate([wb2, jmat], axis=1)

    in_maps = []
    for c in range(CORES):
        r0 = c * NPC
        Xsh = np.zeros((NPAD, D), dtype=np.float32)
        Xsh[:NPC] = Xf[r0 : r0 + NPC]
        # stacked pairs: [f + 64*(t%2), pair*128 + p]
        x4 = Xsh.reshape(NPAIRS, 2, P, D)  # [pair, tpar, p, f]
        xT2 = np.zeros((P, NPAIRS * P + P), dtype=np.float32)
        xT2[:, : NPAIRS * P] = (
            x4.transpose(1, 3, 0, 2).reshape(2 * D, NPAIRS * P)
        )
        xT2[:, NPAIRS * P :] = w2s

        ci_pad = np.zeros((NPAD, DEG), dtype=np.int64)
        ci_pad[:NPC] = ci_all[r0 : r0 + NPC]
        # xgT2[f + 64*tp, pair*2048 + p*16 + k]
        g = Xbf[ci_pad, :]  # [NPAD, DEG, D]
        g5 = g.reshape(NPAIRS, 2, P, DEG, D)  # [pair, tp, p, k, f]
        xgT2 = np.ascontiguousarray(
            g5.transpose(1, 4, 0, 2, 3).reshape(2 * D, NPAIRS * SLOTS)
        )
        in_maps.append(
            {
                "xT2": np.ascontiguousarray(xT2),
                "cst": np.ascontiguousarray(cst),
                "xgT2": xgT2,
            }
        )
    return in_maps


_NC_CACHE = {}


def _get_nc():
    if "nc" not in _NC_CACHE:
        _NC_CACHE["nc"] = build_nc()
    return _NC_CACHE["nc"]


def run(X, weights, attention_w, column_index, trace=False, **trace_kwargs):
    from concourse import bass_utils

    nc = _get_nc()
    in_maps = make_in_maps(X, weights, attention_w, column_index)
    res = bass_utils.run_bass_kernel_spmd(
        nc, in_maps, core_ids=list(range(CORES)), trace=trace, **trace_kwargs
    )
    outs = []
    for c in range(CORES):
        o = np.asarray(res.results[c]["out"])  # [128, NPAIRS*128]
        # out[f + 64*tp, pair*128 + p] -> [node, f]
        o4 = o.reshape(2, D, NPAIRS, P).transpose(2, 0, 3, 1).reshape(NTILES * P, D)
        outs.append(o4[:NPC])
    return np.concatenate(outs, axis=0).astype(np.float32), res


def kernel(
    X,
    weights,
    attention_w,
    row_pointers,
    column_index,
    blockPartition,
    edgeToColumn,
    edgeToRow,
    **_unused,
):
    out, _ = run(X, weights, attention_w, column_index)
    return out


# revision 16
# speedup vs baseline: 1.4281x; 1.0051x over previous
"""AGNNConv (single-head attention message passing) on 8 TRN2 NeuronCores.

Reference computation (N=100000 nodes, fixed degree 16, D=64):
    X_prime = X @ W                                  # [N, 64]
    e[n,k]  = <X_prime[n], X_prime[ci[n,k]]> * s     # s = attention_w[0,0]
    out[n]  = sum_k e[n,k] * X_prime[ci[n,k]]        # [N, 64]

Sharding: nodes split 12500/core across 8 cores, fully independent.
The host pre-gathers raw X rows per edge. Everything on device runs
feature-major with TWO tiles packed on the 128 partitions (tile A's 64
features on partitions 0-63, tile B's on 64-127):

  per tile pair (2 x 128 nodes, 4096 edge slots, slot s = p*16+k):
    D^T   = blockdiag(W,W)^T-matmul over pre-gathered XgT2   (tensor)
    P2    = D^T * xs2 (node features, k-broadcast)           (DVE)
    e_br  = blockdiag(ones)-matmul over P2 -> per-slot dots
            replicated across each tile's 64 partitions      (tensor)
    Qt    = D^T * e_br                                       (GpSimd)
    o^T   = sum_k Qt  (k contiguous innermost)               (DVE)

so the per-edge dot products and their broadcast both ride the idle
tensor engine instead of the vector engine.
"""

import sys

import ml_dtypes
import numpy as np

if "/opt/trn_rl_repo" not in sys.path:
    sys.path.insert(0, "/opt/trn_rl_repo")

N_NODES = 100000
DEG = 16
D = 64
CORES = 8
NPC = N_NODES // CORES  # 12500
P = 128
NTILES = (NPC + P - 1) // P  # 98
NPAIRS = NTILES // 2  # 49
SLOTS = P * DEG  # 2048 slots per tile


def build_nc(lowering=False):
    from concourse import bacc, mybir, tile

    f32 = mybir.dt.float32
    bf16 = mybir.dt.bfloat16

    nc = bacc.Bacc(
        "TRN2", target_bir_lowering=lowering, debug=False, num_devices=CORES
    )

    # xT2: stacked-pair node features [f + 64*(t%2), pair*128 + p] plus the
    # blockdiag(W*s, W*s) stationary in the last 128 columns.
    xT2 = nc.declare_dram_parameter(
        "xT2", [P, NPAIRS * P + P], f32, isOutput=False
    )
    # cst: [Wb2 | J]  (blockdiag(W,W), blockdiag(ones64,ones64)), bf16.
    cst = nc.declare_dram_parameter("cst", [P, 2 * P], bf16, isOutput=False)
    # Pre-gathered neighbor features, stacked-pair feature-major:
    # xgT2[f + 64*(t%2), pair*2048 + p*16 + k] = X[ci[t*128+p, k], f]
    xgT2 = nc.declare_dram_parameter(
        "xgT2", [P, NPAIRS * SLOTS], bf16, isOutput=False
    )
    out_ext = nc.declare_dram_parameter("out", [P, NPAIRS * P], f32, isOutput=True)

    CH = 512  # psum bank chunk (f32)

    with tile.TileContext(nc) as tc:
        with (
            tc.tile_pool(name="const", bufs=1) as cpool,
            tc.tile_pool(name="dps", bufs=4, space="PSUM") as dpsum,
            tc.tile_pool(name="eps", bufs=4, space="PSUM") as epsum,
            tc.tile_pool(name="xg", bufs=4) as xgpool,
            tc.tile_pool(name="db", bufs=3) as dbpool,
            tc.tile_pool(name="p2", bufs=3) as p2pool,
            tc.tile_pool(name="qt", bufs=3) as qtpool,
            tc.tile_pool(name="o", bufs=4) as opool,
        ):
            xT2_sb = cpool.tile([P, NPAIRS * P + P], f32, tag="xT2_sb")
            cst_sb = cpool.tile([P, 2 * P], bf16, tag="cst_sb")
            xs2_sb = cpool.tile([P, NPAIRS * P], bf16, tag="xs2_sb")

            nc.sync.dma_start(out=xT2_sb[:, :], in_=xT2[:, :])
            nc.sync.dma_start(out=cst_sb[:, :], in_=cst[:, :])
            w2s = xT2_sb[:, NPAIRS * P : NPAIRS * P + P]
            wb2 = cst_sb[:, 0:P]
            jj = cst_sb[:, P : 2 * P]

            # Node phase: xs2 = blockdiag(W*s,W*s)^T @ X2T, all pairs first
            # (single stationary load), Act converts PSUM -> bf16.
            for pr in range(NPAIRS):
                ps = dpsum.tile([P, CH], f32, tag="D")
                nc.tensor.matmul(
                    ps[:, 0:P],
                    w2s,
                    xT2_sb[:, pr * P : (pr + 1) * P],
                    start=True,
                    stop=True,
                )
                nc.scalar.copy(
                    out=xs2_sb[:, pr * P : (pr + 1) * P], in_=ps[:, 0:P]
                )

            # Edge phase
            for pr in range(NPAIRS):
                xg = xgpool.tile([P, SLOTS], bf16, tag="xg")
                nc.sync.dma_start(
                    out=xg[:, :], in_=xgT2[:, pr * SLOTS : (pr + 1) * SLOTS]
                )
                Db = dbpool.tile([P, SLOTS], bf16, tag="Db")
                P2 = p2pool.tile([P, SLOTS], bf16, tag="P2")
                Eb = p2pool.tile([P, SLOTS], bf16, tag="Eb")
                Qt = qtpool.tile([P, SLOTS], bf16, tag="Qt")
                o2 = opool.tile([P, P], f32, tag="o2")
                # D^T in 512 bank chunks; Act converts each to bf16 SBUF
                for j in range(4):
                    Dp = dpsum.tile([P, CH], f32, tag="D")
                    nc.tensor.matmul(
                        Dp[:, :],
                        wb2,
                        xg[:, j * CH : (j + 1) * CH],
                        start=True,
                        stop=True,
                    )
                    nc.scalar.copy(
                        out=Db[:, j * CH : (j + 1) * CH], in_=Dp[:, :]
                    )
                # P2 = D * xs2 (bcast over k, 16 inner), full pair width
                nc.vector.tensor_tensor(
                    out=P2[:, :].rearrange("q (p k) -> q p k", k=DEG),
                    in0=Db[:, :].rearrange("q (p k) -> q p k", k=DEG),
                    in1=xs2_sb[:, pr * P : (pr + 1) * P]
                    .unsqueeze(2)
                    .broadcast_to([P, P, DEG]),
                    op=mybir.AluOpType.mult,
                )
                # e_br = blockdiag(ones) @ P2; Act stages to SBUF bf16
                for j in range(4):
                    Ep = epsum.tile([P, CH], f32, tag="E")
                    nc.tensor.matmul(
                        Ep[:, :],
                        jj,
                        P2[:, j * CH : (j + 1) * CH],
                        start=True,
                        stop=True,
                    )
                    nc.scalar.copy(
                        out=Eb[:, j * CH : (j + 1) * CH], in_=Ep[:, :]
                    )
                # Qt = D * e_br full width on GpSimd
                nc.gpsimd.tensor_tensor(
                    out=Qt[:, :],
                    in0=Db[:, :],
                    in1=Eb[:, :],
                    op=mybir.AluOpType.mult,
                )
                nc.vector.tensor_reduce(
                    out=o2[:, :],
                    in_=Qt[:, :].rearrange("q (p k) -> q p k", k=DEG),
                    axis=mybir.AxisListType.X,
                    op=mybir.AluOpType.add,
                )
                nc.sync.dma_start(
                    out=out_ext[:, pr * P : (pr + 1) * P], in_=o2[:, :]
                )

    nc.compile()
    return nc


def make_in_maps(X, weights, attention_w, column_index):
    s = float(np.asarray(attention_w).reshape(-1)[0])
    w = np.asarray(weights, dtype=np.float32)
    Xf = np.asarray(X, dtype=np.float32)
    Xbf = Xf.astype(ml_dtypes.bfloat16)
    ci_all = np.asarray(column_index, dtype=np.int64).reshape(N_NODES, DEG)
    NPAD = NTILES * P

    ws = w * s
    w2s = np.zeros((P, P), dtype=np.float32)
    w2s[0:D, 0:D] = ws
    w2s[D:P, D:P] = ws
    wb2 = np.zeros((P, P), dtype=ml_dtypes.bfloat16)
    wb2[0:D, 0:D] = w.astype(ml_dtypes.bfloat16)
    wb2[D:P, D:P] = w.astype(ml_dtypes.bfloat16)
    jmat = np.zeros((P, P), dtype=ml_dtypes.bfloat16)
    jmat[0:D, 0:D] = 1
    jmat[D:P, D:P] = 1
    cst = np.concatenate([wb2, jmat], axis=1)

    in_maps = []
    for c in range(CORES):
        r0 = c * NPC
        Xsh = np.zeros((NPAD, D), dtype=np.float32)
        Xsh[:NPC] = Xf[r0 : r0 + NPC]
        # stacked pairs: [f + 64*(t%2), pair*128 + p]
        x4 = Xsh.reshape(NPAIRS, 2, P, D)  # [pair, tpar, p, f]
        xT2 = np.zeros((P, NPAIRS * P + P), dtype=np.float32)
        xT2[:, : NPAIRS * P] = (
            x4.transpose(1, 3, 0, 2).reshape(2 * D, NPAIRS * P)
        )
        xT2[:, NPAIRS * P :] = w2s

        ci_pad = np.zeros((NPAD, DEG), dtype=np.int64)
        ci_pad[:NPC] = ci_all[r0 : r0 + NPC]
        # xgT2[f + 64*tp, pair*2048 + p*16 + k]
        g = Xbf[ci_pad, :]  # [NPAD, DEG, D]
        g5 = g.reshape(NPAIRS, 2, P, DEG, D)  # [pair, tp, p, k, f]
        xgT2 = np.ascontiguousarray(
            g5.transpose(1, 4, 0, 2, 3).reshape(2 * D, NPAIRS * SLOTS)
        )
        in_maps.append(
            {
                "xT2": np.ascontiguousarray(xT2),
                "cst": np.ascontiguousarray(cst),
                "xgT2": xgT2,
            }
        )
    return in_maps


_NC_CACHE = {}


def _get_nc():
    if "nc" not in _NC_CACHE:
        _NC_CACHE["nc"] = build_nc()
    return _NC_CACHE["nc"]


def run(X, weights, attention_w, column_index, trace=False, **trace_kwargs):
    from concourse import bass_utils

    nc = _get_nc()
    in_maps = make_in_maps(X, weights, attention_w, column_index)
    res = bass_utils.run_bass_kernel_spmd(
        nc, in_maps, core_ids=list(range(CORES)), trace=trace, **trace_kwargs
    )
    outs = []
    for c in range(CORES):
        o = np.asarray(res.results[c]["out"])  # [128, NPAIRS*128]
        # out[f + 64*tp, pair*128 + p] -> [node, f]
        o4 = o.reshape(2, D, NPAIRS, P).transpose(2, 0, 3, 1).reshape(NTILES * P, D)
        outs.append(o4[:NPC])
    return np.concatenate(outs, axis=0).astype(np.float32), res


def kernel(
    X,
    weights,
    attention_w,
    row_pointers,
    column_index,
    blockPartition,
    edgeToColumn,
    edgeToRow,
    **_unused,
):
    out, _ = run(X, weights, attention_w, column_index)
    return out


# revision 19
# speedup vs baseline: 1.4302x; 1.0015x over previous
"""AGNNConv (single-head attention message passing) on 8 TRN2 NeuronCores.

Reference computation (N=100000 nodes, fixed degree 16, D=64):
    X_prime = X @ W                                  # [N, 64]
    e[n,k]  = <X_prime[n], X_prime[ci[n,k]]> * s     # s = attention_w[0,0]
    out[n]  = sum_k e[n,k] * X_prime[ci[n,k]]        # [N, 64]

Sharding: nodes split 12500/core across 8 cores, fully independent.
The host pre-gathers raw X rows per edge. Everything on device runs
feature-major with TWO tiles packed on the 128 partitions (tile A's 64
features on partitions 0-63, tile B's on 64-127):

  per tile pair (2 x 128 nodes, 4096 edge slots, slot s = p*16+k):
    D^T   = blockdiag(W,W)^T-matmul over pre-gathered XgT2   (tensor)
    P2    = D^T * xs2 (node features, k-broadcast)           (DVE)
    e_br  = blockdiag(ones)-matmul over P2 -> per-slot dots
            replicated across each tile's 64 partitions      (tensor)
    Qt    = D^T * e_br                                       (GpSimd)
    o^T   = sum_k Qt  (k contiguous innermost)               (DVE)

so the per-edge dot products and their broadcast both ride the idle
tensor engine instead of the vector engine.
"""

import sys

import ml_dtypes
import numpy as np

if "/opt/trn_rl_repo" not in sys.path:
    sys.path.insert(0, "/opt/trn_rl_repo")

N_NODES = 100000
DEG = 16
D = 64
CORES = 8
NPC = N_NODES // CORES  # 12500
P = 128
NTILES = (NPC + P - 1) // P  # 98
NPAIRS = NTILES // 2  # 49
SLOTS = P * DEG  # 2048 slots per tile


def build_nc(lowering=False):
    from concourse import bacc, mybir, tile

    f32 = mybir.dt.float32
    bf16 = mybir.dt.bfloat16

    nc = bacc.Bacc(
        "TRN2", target_bir_lowering=lowering, debug=False, num_devices=CORES
    )

    # xT2: stacked-pair node features [f + 64*(t%2), pair*128 + p] plus the
    # blockdiag(W*s, W*s) stationary in the last 128 columns.
    xT2 = nc.declare_dram_parameter(
        "xT2", [P, NPAIRS * P + P], f32, isOutput=False
    )
    # cst: [Wb2 | J]  (blockdiag(W,W), blockdiag(ones64,ones64)), bf16.
    cst = nc.declare_dram_parameter("cst", [P, 2 * P], bf16, isOutput=False)
    # Pre-gathered neighbor features, stacked-pair feature-major:
    # xgT2[f + 64*(t%2), pair*2048 + p*16 + k] = X[ci[t*128+p, k], f]
    xgT2 = nc.declare_dram_parameter(
        "xgT2", [P, NPAIRS * SLOTS], bf16, isOutput=False
    )
    out_ext = nc.declare_dram_parameter("out", [P, NPAIRS * P], f32, isOutput=True)

    CH = 512  # psum bank chunk (f32)

    with tile.TileContext(nc) as tc:
        with (
            tc.tile_pool(name="const", bufs=1) as cpool,
            tc.tile_pool(name="dps", bufs=4, space="PSUM") as dpsum,
            tc.tile_pool(name="eps", bufs=4, space="PSUM") as epsum,
            tc.tile_pool(name="xg", bufs=4) as xgpool,
            tc.tile_pool(name="db", bufs=3) as dbpool,
            tc.tile_pool(name="p2", bufs=3) as p2pool,
            tc.tile_pool(name="qt", bufs=3) as qtpool,
            tc.tile_pool(name="o", bufs=4) as opool,
        ):
            xT2_sb = cpool.tile([P, NPAIRS * P + P], f32, tag="xT2_sb")
            cst_sb = cpool.tile([P, 2 * P], bf16, tag="cst_sb")
            xs2_sb = cpool.tile([P, NPAIRS * P], bf16, tag="xs2_sb")

            nc.sync.dma_start(out=xT2_sb[:, :], in_=xT2[:, :])
            nc.sync.dma_start(out=cst_sb[:, :], in_=cst[:, :])
            w2s = xT2_sb[:, NPAIRS * P : NPAIRS * P + P]
            wb2 = cst_sb[:, 0:P]
            jj = cst_sb[:, P : 2 * P]

            # Node phase: xs2 = blockdiag(W*s,W*s)^T @ X2T, all pairs first
            # (single stationary load), Act converts PSUM -> bf16.
            for pr in range(NPAIRS):
                ps = dpsum.tile([P, CH], f32, tag="D")
                nc.tensor.matmul(
                    ps[:, 0:P],
                    w2s,
                    xT2_sb[:, pr * P : (pr + 1) * P],
                    start=True,
                    stop=True,
                )
                nc.scalar.copy(
                    out=xs2_sb[:, pr * P : (pr + 1) * P], in_=ps[:, 0:P]
                )

            # Edge phase. o/out-DMA are issued one pair late so the DVE's
            # in-order queue never parks on Qt (GpSimd) while the next
            # pair's P2 is ready (software pipelining).
            pending = []

            def flush_pending():
                ppr, pQt = pending.pop(0)
                o2 = opool.tile([P, P], f32, tag="o2")
                nc.vector.tensor_reduce(
                    out=o2[:, :],
                    in_=pQt[:, :].rearrange("q (p k) -> q p k", k=DEG),
                    axis=mybir.AxisListType.X,
                    op=mybir.AluOpType.add,
                )
                nc.sync.dma_start(
                    out=out_ext[:, ppr * P : (ppr + 1) * P], in_=o2[:, :]
                )

            for pr in range(NPAIRS):
                xg = xgpool.tile([P, SLOTS], bf16, tag="xg")
                nc.sync.dma_start(
                    out=xg[:, :], in_=xgT2[:, pr * SLOTS : (pr + 1) * SLOTS]
                )
                Db = dbpool.tile([P, SLOTS], bf16, tag="Db")
                P2 = p2pool.tile([P, SLOTS], bf16, tag="P2")
                Eb = p2pool.tile([P, SLOTS], bf16, tag="Eb")
                Qt = qtpool.tile([P, SLOTS], bf16, tag="Qt")
                # D^T in 512 bank chunks; Act converts each to bf16 SBUF
                for j in range(4):
                    Dp = dpsum.tile([P, CH], f32, tag="D")
                    nc.tensor.matmul(
                        Dp[:, :],
                        wb2,
                        xg[:, j * CH : (j + 1) * CH],
                        start=True,
                        stop=True,
                    )
                    nc.scalar.copy(
                        out=Db[:, j * CH : (j + 1) * CH], in_=Dp[:, :]
                    )
                # P2 = D * xs2 (bcast over k, 16 inner), full pair width
                nc.vector.tensor_tensor(
                    out=P2[:, :].rearrange("q (p k) -> q p k", k=DEG),
                    in0=Db[:, :].rearrange("q (p k) -> q p k", k=DEG),
                    in1=xs2_sb[:, pr * P : (pr + 1) * P]
                    .unsqueeze(2)
                    .broadcast_to([P, P, DEG]),
                    op=mybir.AluOpType.mult,
                )
                # e_br = blockdiag(ones) @ P2; Act stages to SBUF bf16
                for j in range(4):
                    Ep = epsum.tile([P, CH], f32, tag="E")
                    nc.tensor.matmul(
                        Ep[:, :],
                        jj,
                        P2[:, j * CH : (j + 1) * CH],
                        start=True,
                        stop=True,
                    )
                    nc.scalar.copy(
                        out=Eb[:, j * CH : (j + 1) * CH], in_=Ep[:, :]
                    )
                # Qt = D * e_br full width on GpSimd
                nc.gpsimd.tensor_tensor(
                    out=Qt[:, :],
                    in0=Db[:, :],
                    in1=Eb[:, :],
                    op=mybir.AluOpType.mult,
                )
                pending.append((pr, Qt))
                if len(pending) > 1:
                    flush_pending()
            while pending:
                flush_pending()

    nc.compile()
    return nc


def make_in_maps(X, weights, attention_w, column_index):
    s = float(np.asarray(attention_w).reshape(-1)[0])
    w = np.asarray(weights, dtype=np.float32)
    Xf = np.asarray(X, dtype=np.float32)
    Xbf = Xf.astype(ml_dtypes.bfloat16)
    ci_all = np.asarray(column_index, dtype=np.int64).reshape(N_NODES, DEG)
    NPAD = NTILES * P

    ws = w * s
    w2s = np.zeros((P, P), dtype=np.float32)
    w2s[0:D, 0:D] = ws
    w2s[D:P, D:P] = ws
    wb2 = np.zeros((P, P), dtype=ml_dtypes.bfloat16)
    wb2[0:D, 0:D] = w.astype(ml_dtypes.bfloat16)
    wb2[D:P, D:P] = w.astype(ml_dtypes.bfloat16)
    jmat = np.zeros((P, P), dtype=ml_dtypes.bfloat16)
    jmat[0:D, 0:D] = 1
    jmat[D:P, D:P] = 1
    cst = np.concatenate([wb2, jmat], axis=1)

    in_maps = []
    for c in range(CORES):
        r0 = c * NPC
        Xsh = np.zeros((NPAD, D), dtype=np.float32)
        Xsh[:NPC] = Xf[r0 : r0 + NPC]
        # stacked pairs: [f + 64*(t%2), pair*128 + p]
        x4 = Xsh.reshape(NPAIRS, 2, P, D)  # [pair, tpar, p, f]
        xT2 = np.zeros((P, NPAIRS * P + P), dtype=np.float32)
        xT2[:, : NPAIRS * P] = (
            x4.transpose(1, 3, 0, 2).reshape(2 * D, NPAIRS * P)
        )
        xT2[:, NPAIRS * P :] = w2s

        ci_pad = np.zeros((NPAD, DEG), dtype=np.int64)
        ci_pad[:NPC] = ci_all[r0 : r0 + NPC]
        # xgT2[f + 64*tp, pair*2048 + p*16 + k]
        g = Xbf[ci_pad, :]  # [NPAD, DEG, D]
        g5 = g.reshape(NPAIRS, 2, P, DEG, D)  # [pair, tp, p, k, f]
        xgT2 = np.ascontiguousarray(
            g5.transpose(1, 4, 0, 2, 3).reshape(2 * D, NPAIRS * SLOTS)
        )
        in_maps.append(
            {
                "xT2": np.ascontiguousarray(xT2),
                "cst": np.ascontiguousarray(cst),
                "xgT2": xgT2,
            }
        )
    return in_maps


_NC_CACHE = {}


def _get_nc():
    if "nc" not in _NC_CACHE:
        _NC_CACHE["nc"] = build_nc()
    return _NC_CACHE["nc"]


def run(X, weights, attention_w, column_index, trace=False, **trace_kwargs):
    from concourse import bass_utils

    nc = _get_nc()
    in_maps = make_in_maps(X, weights, attention_w, column_index)
    res = bass_utils.run_bass_kernel_spmd(
        nc, in_maps, core_ids=list(range(CORES)), trace=trace, **trace_kwargs
    )
    outs = []
    for c in range(CORES):
        o = np.asarray(res.results[c]["out"])  # [128, NPAIRS*128]
        # out[f + 64*tp, pair*128 + p] -> [node, f]
        o4 = o.reshape(2, D, NPAIRS, P).transpose(2, 0, 3, 1).reshape(NTILES * P, D)
        outs.append(o4[:NPC])
    return np.concatenate(outs, axis=0).astype(np.float32), res


def kernel(
    X,
    weights,
    attention_w,
    row_pointers,
    column_index,
    blockPartition,
    edgeToColumn,
    edgeToRow,
    **_unused,
):
    out, _ = run(X, weights, attention_w, column_index)
    return out
